# revision 7
# baseline (speedup 1.0000x reference)
"""fp8 variant v2: all engine ops are 2D (the BIR verifier rejects 3D BNStats).

Stream W as scaled float8_e4m3.  Per tile of [128 classes, 1024 d]:
  - PE gram on the first NPE_A=5 (chain A) / NPE_B=2 (chain B tail) d-groups
    + the ones-moving rowsum trick for all 8 groups.
  - The remaining columns are assigned per-tile-contiguous to one engine:
    A-chunks (S=4): tile0-rest -> ACT square+accum, tiles1,2-rest -> DVE
    bn_stats, tile3-rest -> Pool square+XYZWC reduce.
    B-chunks (S=2): tile0-rest -> ACT, tile1-rest -> DVE (2 ops).
    Last chunk (S=1): rest -> DVE (2 ops; lowest tail latency).
Host math identical to kernel_fp8 (scale 16, diag of gram banks, bn_stats
even/odd count/mean/M2 recombination).
"""

import numpy as np

D = 1024
N_CLASSES = 50000
N_CORES = 8
P = 128
CPC_RAW = N_CLASSES // N_CORES
T_TILES = (CPC_RAW + P - 1) // P        # 49
CPC = T_TILES * P                       # 6272
G = D // P                              # 8
SCALE = 16.0

CHUNKS = [4] * 11 + [2, 2, 1]           # 49 tiles
assert sum(CHUNKS) == T_TILES
GRAM_A_TILES = 44                       # chain A: tiles 0..43 (chunks 0..10)
NPE_A = 5
NPE_B = 2
BULK_A_CHUNKS = 9

LAST_RESULTS = None
_NC_CACHE = {}
_COLMAP = {}


def _build_bass(bufs=14):
    import concourse.mybir as mybir
    from concourse import bacc
    from concourse.tile import TileContext

    nc = bacc.Bacc(
        "TRN2", target_bir_lowering=False, debug=False, num_devices=N_CORES
    )
    f32 = mybir.dt.float32
    fp8 = mybir.dt.float8e4
    bf16 = mybir.dt.bfloat16

    wt = nc.declare_dram_parameter("wt", [CPC, D], fp8, isOutput=False)
    last_ci = len(CHUNKS) - 1

    # per-chunk op lists: (kind, tile_s, off, width) with off/width inside the
    # tile's 1024 cols; all 2D-contiguous
    def chunk_ops(ci, S):
        if ci <= 10:                       # A-chunks, S == 4
            b = NPE_A * P                  # 640
            return ([("act", 0, b, 1024 - b)],
                    [("dve", 1, b, 1024 - b), ("dve", 2, b, 1024 - b)],
                    [("pool", 3, b, 1024 - b)])
        b = NPE_B * P                      # 256
        if S == 2:
            return ([("act", 0, b, 1024 - b)],
                    [("dve", 1, b, 512), ("dve", 1, b + 512, 256)],
                    [])
        return ([],
                [("dve", 0, b, 512), ("dve", 0, b + 512, 256)],
                [])

    # ---- stats layout: bulk = chunks 0..8 + gram A; tail = rest + gram B + rs
    acts, dves, pools = [], [], []
    col = 0

    def emit_cols(ci, S):
        nonlocal col
        a_ops, d_ops, p_ops = chunk_ops(ci, S)
        for _ in a_ops:
            acts.append((ci, col)); col += 1
        for _ in d_ops:
            dves.append((ci, col)); col += 6
        for _ in p_ops:
            pools.append((ci, col)); col += 1

    for ci in range(BULK_A_CHUNKS):
        emit_cols(ci, CHUNKS[ci])
    gram_a_col = col
    col += NPE_A * P
    bulk_len = col
    for ci in range(BULK_A_CHUNKS, len(CHUNKS)):
        emit_cols(ci, CHUNKS[ci])
    gram_b_col = col
    col += NPE_B * P
    rs_col = col
    col += G
    nslot = col
    _COLMAP.update(act=acts, dve=dves, pool=pools, gram_a=gram_a_col,
                   gram_b=gram_b_col, rs=rs_col, bulk_len=bulk_len,
                   nslot=nslot)

    out = nc.declare_dram_parameter("stats", [P, nslot], f32, isOutput=True)

    with TileContext(nc) as tc:
        with (
            tc.tile_pool(name="wpool", bufs=bufs) as wpool,
            tc.tile_pool(name="spool", bufs=1) as spool,
            tc.tile_pool(name="scpool", bufs=1) as scpool,
            tc.tile_pool(name="cpool", bufs=1) as cpool,
            tc.tile_pool(name="pspool", bufs=1, space="PSUM") as pspool,
        ):
            stats = spool.tile([P, nslot], f32)
            scratch = scpool.tile([P, 1024], bf16)
            pscratch = scpool.tile([P, 512], bf16)
            ones = cpool.tile([P, 1], fp8)
            nc.gpsimd.memset(ones, 1.0)
            rs_psum = pspool.tile([P, G], f32)
            gram_a = pspool.tile([P, NPE_A * P], f32)
            gram_b = pspool.tile([P, NPE_B * P], f32)

            a_idx = d_idx = p_idx = 0
            tile0 = 0
            for ci, S in enumerate(CHUNKS):
                ctile_full = wpool.tile([P, 4096], fp8, tag="wtile")
                ctile = ctile_full[:, :S * 1024]
                src = wt[tile0 * P:(tile0 + S) * P, :].rearrange(
                    "(s p) d -> p s d", p=P
                )
                nc.sync.dma_start(out=ctile, in_=src)
                if ci == last_ci:
                    nc.sync.dma_start(
                        out=out[:, :bulk_len], in_=stats[:, :bulk_len]
                    )

                for s in range(S):
                    t_glob = tile0 + s
                    for g in range(G):
                        nc.tensor.matmul(
                            rs_psum[:, g:g + 1],
                            ctile[:, s * 1024 + g * P: s * 1024 + (g + 1) * P],
                            ones,
                            start=(t_glob == 0),
                            stop=(t_glob == T_TILES - 1),
                        )
                    in_a = t_glob < GRAM_A_TILES
                    gp = gram_a if in_a else gram_b
                    gstart = (t_glob == 0) if in_a else (t_glob == GRAM_A_TILES)
                    gstop = (t_glob == GRAM_A_TILES - 1) if in_a else (
                        t_glob == T_TILES - 1)
                    for g in range(NPE_A if in_a else NPE_B):
                        blk = ctile[:, s * 1024 + g * P: s * 1024 + (g + 1) * P]
                        nc.tensor.matmul(
                            gp[:, g * P:(g + 1) * P], blk, blk,
                            start=gstart, stop=gstop,
                        )

                a_ops, d_ops, p_ops = chunk_ops(ci, S)
                for _, s, off, wdt in d_ops:
                    _, c0 = dves[d_idx]; d_idx += 1
                    nc.vector.bn_stats(
                        stats[:, c0:c0 + 6],
                        ctile[:, s * 1024 + off: s * 1024 + off + wdt],
                    )
                for _, s, off, wdt in p_ops:
                    _, c0 = pools[p_idx]; p_idx += 1
                    sl = ctile[:, s * 1024 + off: s * 1024 + off + wdt]
                    nc.gpsimd.tensor_tensor(
                        out=pscratch[:, :wdt], in0=sl, in1=sl,
                        op=mybir.AluOpType.mult,
                    )
                    nc.gpsimd.reduce_sum(
                        stats[0:1, c0:c0 + 1], pscratch[:, :wdt],
                        axis=mybir.AxisListType.XYZWC,
                    )
                for _, s, off, wdt in a_ops:
                    _, c0 = acts[a_idx]; a_idx += 1
                    nc.scalar.activation(
                        scratch[:, :wdt],
                        ctile[:, s * 1024 + off: s * 1024 + off + wdt],
                        mybir.ActivationFunctionType.Square,
                        accum_out=stats[:, c0:c0 + 1],
                    )

                if ci == 10:
                    nc.scalar.copy(
                        out=stats[:, gram_a_col:gram_a_col + NPE_A * P],
                        in_=gram_a,
                    )
                tile0 += S

            nc.scalar.copy(
                out=stats[:, gram_b_col:gram_b_col + NPE_B * P], in_=gram_b
            )
            nc.vector.tensor_scalar_add(
                stats[:, rs_col:rs_col + G], rs_psum, 0.0
            )
            nc.sync.dma_start(out=out[:, bulk_len:], in_=stats[:, bulk_len:])
    nc.compile()
    return nc


def kernel(softmax_weight, group_ids=None, batch_size=32, **_ignored):
    global LAST_RESULTS
    import ml_dtypes
    from concourse.bass_utils import run_bass_kernel_spmd

    W = np.asarray(softmax_weight, dtype=np.float32)
    assert W.shape == (D, N_CLASSES), W.shape
    bs = float(np.asarray(batch_size))

    if "nc" not in _NC_CACHE:
        _NC_CACHE["nc"] = _build_bass()
    nc = _NC_CACHE["nc"]

    f8 = ml_dtypes.float8_e4m3
    in_maps = []
    for k in range(N_CORES):
        shard = np.zeros((CPC, D), dtype=f8)
        shard[:CPC_RAW] = (W[:, k * CPC_RAW:(k + 1) * CPC_RAW].T * SCALE).astype(f8)
        in_maps.append({"wt": shard})
    LAST_RESULTS = run_bass_kernel_spmd(nc, in_maps, core_ids=list(range(N_CORES)))

    om = 0.0
    t = np.zeros(D, np.float64)
    cm = _COLMAP
    for r in LAST_RESULTS.results:
        st = np.asarray(r["stats"]).astype(np.float64)
        for _, c0 in cm["act"]:
            om += st[:, c0].sum()
        for _, c0 in cm["pool"]:
            om += st[0, c0]
        for _, c0 in cm["dve"]:
            grp = st[:, c0:c0 + 6]
            om += (grp[:, 2] + grp[:, 0] * grp[:, 1] ** 2).sum()
            om += (grp[:, 5] + grp[:, 3] * grp[:, 4] ** 2).sum()
        for gcol, npe in ((cm["gram_a"], NPE_A), (cm["gram_b"], NPE_B)):
            for g in range(npe):
                om += np.diag(st[:, gcol + g * P:gcol + (g + 1) * P]).sum()
        for g in range(G):
            t[g * P:(g + 1) * P] += st[:, cm["rs"] + g]

    om /= SCALE * SCALE
    t /= SCALE
    T = (t @ t) / N_CLASSES
    loss = om + 0.5 * (om - T) / bs
    return np.asarray(loss, dtype=np.float32)
